# revision 1
# baseline (speedup 1.0000x reference)
"""BitSwarmLinear Trainium2 kernel.

Computation (reference):
    swarm_sum = population.sum(axis=2)          # (out, in)
    w_eff     = sign(swarm_sum), sign(0) -> +1  # (out, in), +-1
    y         = einsum("bsi,oi->bso", x, w_eff) # (4, 4096, out)

Distribution (8 NeuronCores, tensor-parallel on out_features):
    - population sharded on out_features: each core gets its 256 rows,
      reduces + binarizes them and computes its 256 output columns.
    - x replicated to every core, staged pre-transposed/tiled as bf16 so the
      contraction dim lands on SBUF partitions with fully-contiguous DMA.
    - outputs gathered on the host along the feature dim.

Host staging (lossless / layout-only):
    - population is exactly +-1.0 -> int8, rearranged swarm-major
      [32, out_c, in]: cuts the dominant input stream 4x and lets the DMA
      engines' inline CCE ALU do the swarm reduction during transfer.
    - x -> bf16 x^T, tiled [tb, 128 ki, 16 ko, TB tok] so every DMA line is
      a 32KB contiguous run (line-rate HBM).
    - y comes back bf16 tile-major; host restores [b, s, out] f32.

Per-core device pipeline:
    1. Four parallel SWDGE accumulate chains (8 DMAs each, CCE int8 add)
       reduce the swarm axis while transferring; DVE merges 4 partials,
       binarizes via (s >= 0) * 2 - 1 (exact: sums are even ints, 0 -> +1).
    2. PE-transpose the sign matrix into W [in(part), out] bf16 (SBUF
       resident, 1 MB).
    3. Stream x^T tiles (4MB contiguous DMAs, deep prefetch); per 128-token
       block accumulate 16 K-tile matmuls into PSUM [128 tok, 256 out]
       (fp32), round to bf16, store on the scalar HWDGE ring.
"""

import os
import sys

import numpy as np

for _p in ("/root/.axon_site/_ro/trn_rl_repo", "/opt/trn_rl_repo"):
    if os.path.isdir(_p) and _p not in sys.path:
        sys.path.append(_p)

import ml_dtypes

# bass_utils' axon trace path imports antenv.axon_hooks, which this image
# lacks. Provide it (backed by the ctypes NTFF hook) so running with
# BASS_TRACE=1 works instead of crashing on the import.
try:
    import antenv.axon_hooks  # noqa: F401
except ImportError:
    try:
        import types as _types

        from trn_agent_boot.trn_boot import _ntff_profile_via_ctypes

        _hooks = _types.ModuleType("antenv.axon_hooks")
        _ntff_hook = _ntff_profile_via_ctypes("/opt/axon/libaxon_pjrt.so")
        _hooks.get_axon_ntff_profile_hook = lambda: _ntff_hook
        _hooks.set_axon_ntff_profile_hook = lambda h: None
        sys.modules["antenv.axon_hooks"] = _hooks
    except Exception:
        pass

import concourse.bass as bass  # noqa: F401  (AP helpers)
import concourse.mybir as mybir
import concourse.tile as tile
from concourse import bacc
from concourse.bass_utils import run_bass_kernel_spmd
from concourse.masks import make_identity

P = 128
IN_F = 2048
SWARM = 32
OUT_F = 2048
N_CORES = 8
OUT_C = OUT_F // N_CORES  # 256 out features per core
TOKENS = 4 * 4096

F32 = mybir.dt.float32
BF16 = mybir.dt.bfloat16
U8 = mybir.dt.uint8
I16 = mybir.dt.int16

# token-block per x^T DMA / output store
TB = 1024
# x^T prefetch depth (SBUF: 32KB/partition each at TB=1024)
XT_BUFS = 4
# swarm-slice staging depth for the reduction
POP_BUFS = 4


def build_nc(tokens: int = TOKENS, out_c: int = OUT_C, in_f: int = IN_F,
             reps: int = 1):
    """Build the per-core Bass program (same program on all 8 cores).

    reps>1 repeats the whole pipeline back-to-back (timing harness only)."""
    ko_tiles = in_f // P          # 16 K-tiles
    oc_groups = out_c // P        # 2 groups of 128 out rows
    tb_count = tokens // TB
    m_per_tb = TB // P

    nc = bacc.Bacc(
        "TRN2",
        target_bir_lowering=False,
        debug=False,
        enable_asserts=False,
        num_devices=N_CORES,
    )

    xT = nc.dram_tensor("xT", [tb_count, P, ko_tiles, TB], BF16,
                        kind="ExternalInput")
    pop = nc.dram_tensor("pop", [SWARM, out_c, in_f], U8,
                         kind="ExternalInput")
    y = nc.dram_tensor("y", [tb_count, P, m_per_tb, out_c], BF16,
                       kind="ExternalOutput")

    xr = xT.ap()                                              # [tb,128,ko,TB]
    pr = pop.ap().rearrange("s (g p) i -> s p g i", p=P)      # [32,128,oc,in]
    yr = y.ap()                                               # [tb,128,m,oc*P]

    with tile.TileContext(nc) as tc:
        with (
            tc.tile_pool(name="const", bufs=1) as const_pool,
            tc.tile_pool(name="pops", bufs=POP_BUFS) as pop_pool,
            tc.tile_pool(name="acc", bufs=1) as acc_pool,
            tc.tile_pool(name="sgn", bufs=oc_groups) as sgn_pool,
            tc.tile_pool(name="wsb", bufs=1) as w_pool,
            tc.tile_pool(name="xt", bufs=XT_BUFS) as x_pool,
            tc.tile_pool(name="ystage", bufs=2) as y_pool,
            tc.tile_pool(name="psum_t", bufs=2, space="PSUM") as psum_t_pool,
            tc.tile_pool(name="psum_y", bufs=4, space="PSUM") as psum_y_pool,
        ):
            ident = const_pool.tile([P, P], F32)
            make_identity(nc, ident[:])

            for _rep in range(reps):
                _emit_body(
                    nc, ident, w_pool, pop_pool, acc_pool, sgn_pool, x_pool,
                    y_pool, psum_t_pool, psum_y_pool, pr, xr, yr,
                    oc_groups, ko_tiles, tb_count, m_per_tb, out_c, in_f,
                )

    nc.compile()  # bacc register allocation / DCE — required before codegen
    return nc


def _emit_body(nc, ident, w_pool, pop_pool, acc_pool, sgn_pool, x_pool,
               y_pool, psum_t_pool, psum_y_pool, pr, xr, yr,
               oc_groups, ko_tiles, tb_count, m_per_tb, out_c, in_f):
    # W in [in(part), ko, out] bf16 — matmul rhs tiles, SBUF-resident
    w_sb = w_pool.tile([P, ko_tiles, out_c], BF16, tag="wsb")

    # ---- Stage 1: swarm reduction as packed-byte adds.
    # pop is staged {0,1} uint8; 2 bytes are summed per int16 lane — no
    # carries cross byte lanes (every lane stays in [0, 32]), and int16
    # values <= 8224 survive the DVE's fp32 ALU exactly.
    acc = acc_pool.tile([P, oc_groups, in_f // 2], I16, tag="acc")
    for s in range(SWARM):
        pt = pop_pool.tile([P, oc_groups, in_f], U8, tag="pops")
        eng = nc.sync if s % 2 == 0 else nc.scalar
        eng.dma_start(pt[:], pr[s])
        if s == 0:
            nc.vector.tensor_copy(out=acc[:], in_=pt[:].bitcast(I16))
        else:
            nc.vector.tensor_add(acc[:], acc[:], pt[:].bitcast(I16))

    # ---- Stage 2: binarize + PE-transpose into W [in, out] bf16
    acc_u8 = acc[:].bitcast(U8)  # [128, oc, in] counts in [0, 32]
    for oc in range(oc_groups):
        sgn = sgn_pool.tile([P, in_f], F32, tag="sgn", name=f"sgn{oc}")
        # count >= 16  <=>  swarm_sum >= 0; w = (count >= 16) * 2 - 1
        nc.vector.tensor_scalar(
            out=sgn[:], in0=acc_u8[:, oc, :], scalar1=16, scalar2=2.0,
            op0=mybir.AluOpType.is_ge, op1=mybir.AluOpType.mult,
        )
        nc.vector.tensor_scalar(
            out=sgn[:], in0=sgn[:], scalar1=1.0, scalar2=None,
            op0=mybir.AluOpType.subtract,
        )
        for k in range(ko_tiles):
            pt_ps = psum_t_pool.tile([P, P], F32, tag="tps")
            nc.tensor.transpose(
                pt_ps[:], sgn[:, k * P : (k + 1) * P], ident[:]
            )
            nc.vector.tensor_copy(
                out=w_sb[:, k, oc * P : (oc + 1) * P], in_=pt_ps[:]
            )

    # ---- Stage 3: stream x^T, matmul, store y (bf16)
    for tb in range(tb_count):
        xt = x_pool.tile([P, ko_tiles, TB], BF16, tag="xt")
        nc.sync.dma_start(xt[:], xr[tb])
        ystage = y_pool.tile([P, m_per_tb, out_c], BF16, tag="ys")
        for m in range(m_per_tb):
            ps = psum_y_pool.tile([P, out_c], F32, tag="yps")
            for k in range(ko_tiles):
                nc.tensor.matmul(
                    ps[:],
                    xt[:, k, m * P : (m + 1) * P],
                    w_sb[:, k, :],
                    start=(k == 0),
                    stop=(k == ko_tiles - 1),
                )
            nc.vector.tensor_copy(out=ystage[:, m, :], in_=ps[:])
        # stores ride the ACT HWDGE ring; loads own the SP ring
        nc.scalar.dma_start(yr[tb], ystage[:])


_NC_CACHE: dict = {}


def _get_nc(tokens=TOKENS, out_c=OUT_C, in_f=IN_F):
    key = (tokens, out_c, in_f)
    if key not in _NC_CACHE:
        _NC_CACHE[key] = build_nc(*key)
    return _NC_CACHE[key]


def stage_x(x: np.ndarray, tokens: int, in_f: int):
    """x [b, s, in] f32 -> tiled bf16 [tb, 128 ki, ko, TB] of x^T."""
    xb = np.ascontiguousarray(
        x.reshape(tokens, in_f).T
    ).astype(ml_dtypes.bfloat16)  # [in, tokens]
    ko = in_f // P
    tb = tokens // TB
    # (ko ki) (tb t) -> tb ki ko t
    return np.ascontiguousarray(
        xb.reshape(ko, P, tb, TB).transpose(2, 1, 0, 3)
    )


def stage_pop_slice(pop_c: np.ndarray):
    """pop slice [out_c, in, 32] (+-1.0 f32) -> swarm-major {0,1} uint8
    [32, out_c, in]. Lossless recode: -1 -> 0, +1 -> 1."""
    return np.ascontiguousarray(
        (pop_c > 0).astype(np.uint8).transpose(2, 0, 1)
    )


def unstage_y(y_dev: np.ndarray, tokens: int, out_c: int):
    """y [tb, 128 p, m, out_c] bf16 -> [tokens, out_c] f32
    (token = tb*TB + m*128 + p)."""
    return (
        y_dev.astype(np.float32)
        .transpose(0, 2, 1, 3)
        .reshape(tokens, out_c)
    )


def prep_inputs(x: np.ndarray, population: np.ndarray):
    tokens = x.shape[0] * x.shape[1]
    in_f = x.shape[2]
    xT = stage_x(x, tokens, in_f)
    out_c = population.shape[0] // N_CORES
    in_maps = []
    for c in range(N_CORES):
        pop_c = stage_pop_slice(population[c * out_c : (c + 1) * out_c])
        in_maps.append({"xT": xT, "pop": pop_c})
    return in_maps, tokens, out_c, in_f


def kernel(x: np.ndarray, population: np.ndarray):
    in_maps, tokens, out_c, in_f = prep_inputs(x, population)
    nc = _get_nc(tokens, out_c, in_f)
    res = run_bass_kernel_spmd(nc, in_maps, core_ids=list(range(N_CORES)))
    y_full = np.concatenate(
        [unstage_y(r["y"], tokens, out_c) for r in res.results], axis=1
    )
    return y_full.reshape(x.shape[0], x.shape[1], population.shape[0])



# revision 6
# speedup vs baseline: 1.0455x; 1.0455x over previous
"""BitSwarmLinear Trainium2 kernel (v2: 2D-sharded, nibble-packed population).

Computation (reference):
    swarm_sum = population.sum(axis=2)          # (out, in)
    w_eff     = sign(swarm_sum), sign(0) -> +1  # (out, in), +-1
    y         = einsum("bsi,oi->bso", x, w_eff) # (4, 4096, out)

Distribution (8 NeuronCores, 4-way out_features x 2-way tokens):
    core c: oc_shard = c % 4 (512 out rows), tok_shard = c // 4 (8192 tokens).
    Per-core HBM traffic: x^T 33.5MB bf16 + pop 16.8MB nibble-packed +
    y 8.4MB bf16 ~= 59MB, well under the ~220us of tensor-engine work, so
    the kernel is compute-bound (the v1 kernel shipped 92MB/core and was
    DMA-bound at ~366us).

Host staging (lossless / layout-only):
    - population +-1.0 -> {0,1} bits; two swarm members packed per byte
      (lo/hi nibble) -> 16 byte-planes. Bijective recode, no arithmetic.
    - x -> bf16 x^T [128 in-part, 16 ko, 8192 tok] (32KB lines).
    - y returns [4 q, 128 oc-part, 8192 tok] bf16.

Per-core device pipeline (every engine has one role):
    - gpsimd (SWDGE ring): streams the 16 pop tiles, one per (q, kb).
    - DVE: swarm reduction. Planes 0-14 summed in u16 lanes (per-byte sums
      <= 15*17=255: no cross-byte carries). Nibble decode via exact fp32
      magic-number floor: H = rne((S15+24.5)/16 + 2^23) - (2^23+2), then
      count = S15 - 15H + (P15 - 15*[P15>=16]); w = 2*[count>=16]-1 in bf16.
    - PE: W^T tiles stationary [128 in, 128 oc], x^T moving [128 in, 512 tok],
      accumulating over 16 K-tiles into PSUM [128 oc, 512 tok]. Token groups
      of 6/6/4 banks; W stays SBUF-resident so x streams exactly once.
    - ACT (qAct HWDGE): PSUM->SBUF bf16 casts + y stores.
    - sync (qSP HWDGE): x chunk loads (16 x 512-token chunks, ring of 8).
"""

import os
import sys

import numpy as np

for _p in ("/root/.axon_site/_ro/trn_rl_repo", "/opt/trn_rl_repo"):
    if os.path.isdir(_p) and _p not in sys.path:
        sys.path.append(_p)

import ml_dtypes

# bass_utils' axon trace path imports antenv.axon_hooks, which this image
# lacks. Provide it (backed by the ctypes NTFF hook) so running with
# BASS_TRACE=1 works instead of crashing on the import.
try:
    import antenv.axon_hooks  # noqa: F401
except ImportError:
    try:
        import types as _types

        from trn_agent_boot.trn_boot import _ntff_profile_via_ctypes

        _hooks = _types.ModuleType("antenv.axon_hooks")
        _ntff_hook = _ntff_profile_via_ctypes("/opt/axon/libaxon_pjrt.so")
        _hooks.get_axon_ntff_profile_hook = lambda: _ntff_hook
        _hooks.set_axon_ntff_profile_hook = lambda h: None
        sys.modules["antenv.axon_hooks"] = _hooks
    except Exception:
        pass

import concourse.mybir as mybir
import concourse.tile as tile
from concourse import bacc
from concourse.bass_utils import run_bass_kernel_spmd

P = 128
IN_F = 2048
OUT_F = 2048
SWARM = 32
TOKENS = 4 * 4096
N_CORES = 8

OC_SHARDS = 4            # out_features shards
TOK_SHARDS = 2           # token shards
OUT_C = OUT_F // OC_SHARDS      # 512 out features per core
TOK_C = TOKENS // TOK_SHARDS    # 8192 tokens per core

KO = IN_F // P           # 16 K-tiles
KB = 4                   # k-blocks (4 K-tiles each) for the reduction
JP = SWARM // 2          # 16 nibble byte-planes
Q = OUT_C // P           # 4 oc blocks of 128

CHUNK = 512              # tokens per x chunk / PSUM bank
N_CHUNKS = TOK_C // CHUNK        # 16
X_BUFS = 8
GROUPS = [list(range(0, 6)), list(range(6, 12)), list(range(12, 16))]

F32 = mybir.dt.float32
BF16 = mybir.dt.bfloat16
U8 = mybir.dt.uint8
U16 = mybir.dt.uint16

MAGIC = 8388608.0        # 2^23
ALU = mybir.AluOpType
ACTF = mybir.ActivationFunctionType


def build_nc():
    nc = bacc.Bacc(
        "TRN2",
        target_bir_lowering=False,
        debug=False,
        enable_asserts=False,
        num_devices=N_CORES,
    )

    xT = nc.dram_tensor("xT", [P, KO, TOK_C], BF16, kind="ExternalInput")
    pop = nc.dram_tensor("pop", [Q, KB, P, JP, KO // KB * P], U8,
                         kind="ExternalInput")
    y = nc.dram_tensor("y", [Q, P, TOK_C], BF16, kind="ExternalOutput")

    xr = xT.ap()
    pr = pop.ap()
    yr = y.ap()

    with tile.TileContext(nc) as tc:
        with (
            tc.tile_pool(name="wsb", bufs=1) as w_pool,
            tc.tile_pool(name="xc", bufs=X_BUFS) as x_pool,
            tc.tile_pool(name="popt", bufs=4) as pop_pool,
            tc.tile_pool(name="acc", bufs=2) as acc_pool,
            tc.tile_pool(name="tmp", bufs=2) as tmp_pool,
            tc.tile_pool(name="ys", bufs=2) as ys_pool,
            tc.tile_pool(name="psum", bufs=8, space="PSUM") as psum_pool,
        ):
            w_sb = w_pool.tile([P, KO, OUT_C], BF16, tag="wsb")

            # ---- swarm reduction + binarize: per (q, kb) unit.
            # DVE only; pop DMA rides the gpsimd SWDGE ring so the paced
            # trigger stream can't block the ACT/sync HWDGE rings.
            for q in range(Q):
                for kb in range(KB):
                    popt = pop_pool.tile([P, JP, 512], U8, tag="popt")
                    nc.gpsimd.dma_start(popt[:], pr[q, kb])

                    acc16 = acc_pool.tile([P, 256], U16, tag="acc")
                    nc.vector.tensor_copy(
                        out=acc16[:], in_=popt[:, 0, :].bitcast(U16))
                    for j in range(1, JP - 1):
                        nc.vector.tensor_add(
                            acc16[:], acc16[:], popt[:, j, :].bitcast(U16))

                    s15 = tmp_pool.tile([P, 512], F32, tag="s15")
                    nc.vector.tensor_copy(out=s15[:], in_=acc16[:].bitcast(U8))
                    # H = floor(S15/16) via exact fp32 magic rounding; h ends
                    # as -15*H (in-place chain)
                    h = tmp_pool.tile([P, 512], F32, tag="h")
                    nc.vector.tensor_scalar(
                        out=h[:], in0=s15[:], scalar1=24.5, scalar2=0.0625,
                        op0=ALU.add, op1=ALU.mult)
                    nc.vector.tensor_scalar(
                        out=h[:], in0=h[:], scalar1=MAGIC, scalar2=None,
                        op0=ALU.add)
                    nc.vector.tensor_scalar(
                        out=h[:], in0=h[:], scalar1=MAGIC + 2.0,
                        scalar2=-15.0, op0=ALU.subtract, op1=ALU.mult)
                    # e = P15 - 15*[P15>=16]  (= lo15+hi15 of plane 15)
                    e = tmp_pool.tile([P, 512], F32, tag="e")
                    nc.vector.tensor_scalar(
                        out=e[:], in0=popt[:, JP - 1, :], scalar1=16,
                        scalar2=-15.0, op0=ALU.is_ge, op1=ALU.mult)
                    pf = tmp_pool.tile([P, 512], F32, tag="pf")
                    nc.vector.tensor_copy(out=pf[:], in_=popt[:, JP - 1, :])
                    nc.vector.tensor_add(e[:], pf[:], e[:])
                    # count = S15 - 15H + e, then w0 = 2*[count>=16]
                    nc.vector.tensor_add(s15[:], s15[:], h[:])
                    nc.vector.tensor_add(s15[:], s15[:], e[:])
                    nc.vector.tensor_scalar(
                        out=s15[:], in0=s15[:], scalar1=16.0, scalar2=2.0,
                        op0=ALU.is_ge, op1=ALU.mult)
                    # w = w0 - 1, straight into the bf16 W tile
                    for kl in range(KO // KB):
                        nc.vector.tensor_scalar(
                            out=w_sb[:, kb * 4 + kl, q * P:(q + 1) * P],
                            in0=s15[:, kl * P:(kl + 1) * P],
                            scalar1=1.0, scalar2=None, op0=ALU.subtract)

            # ---- x chunk loads (sync HWDGE ring only carries these)
            xc_tiles = []
            for c in range(N_CHUNKS):
                xc = x_pool.tile([P, KO, CHUNK], BF16, tag="xc")
                nc.sync.dma_start(xc[:], xr[:, :, c * CHUNK:(c + 1) * CHUNK])
                xc_tiles.append(xc)

            # ---- matmul passes; casts + y stores on the ACT ring
            for g, chunks in enumerate(GROUPS):
                nb = len(chunks)
                for q in range(Q):
                    banks = [psum_pool.tile([P, CHUNK], F32, tag="ps",
                                            name=f"ps{g}_{q}_{b}")
                             for b in range(nb)]
                    for k in range(KO):
                        lhsT = w_sb[:, k, q * P:(q + 1) * P]
                        for b, c in enumerate(chunks):
                            nc.tensor.matmul(
                                banks[b][:], lhsT, xc_tiles[c][:, k, :],
                                start=(k == 0), stop=(k == KO - 1))
                    ys = ys_pool.tile([P, 6 * CHUNK], BF16, tag="ys")
                    for b, c in enumerate(chunks):
                        nc.scalar.activation(
                            out=ys[:, b * CHUNK:(b + 1) * CHUNK],
                            in_=banks[b][:], func=ACTF.Copy)
                    t0 = chunks[0] * CHUNK
                    nc.scalar.dma_start(
                        yr[q][:, t0:t0 + nb * CHUNK], ys[:, :nb * CHUNK])

    nc.compile()
    return nc


_NC_CACHE: dict = {}


def _get_nc():
    if "nc" not in _NC_CACHE:
        _NC_CACHE["nc"] = build_nc()
    return _NC_CACHE["nc"]


def stage_x_half(xf: np.ndarray, th: int):
    """x tokens slice -> bf16 x^T tiled [128, KO, TOK_C]."""
    xh = np.ascontiguousarray(
        xf[th * TOK_C:(th + 1) * TOK_C].T
    ).astype(ml_dtypes.bfloat16)            # [in, tok]
    return np.ascontiguousarray(
        xh.reshape(KO, P, TOK_C).transpose(1, 0, 2))


def stage_pop_shard(pop_sl: np.ndarray):
    """pop slice [512, in, 32] (+-1.0) -> nibble-packed [Q, KB, P, JP, 512]."""
    pb = (pop_sl > 0).astype(np.uint8)                       # {0,1}
    pl = pb[..., 0::2] + 16 * pb[..., 1::2]                  # [512, in, JP]
    st = pl.reshape(Q, P, KB, KO // KB, P, JP)
    return np.ascontiguousarray(
        st.transpose(0, 2, 4, 5, 3, 1).reshape(Q, KB, P, JP, 512))


def prep_inputs(x: np.ndarray, population: np.ndarray):
    xf = x.reshape(TOKENS, IN_F)
    x_halves = [stage_x_half(xf, th) for th in range(TOK_SHARDS)]
    pop_shards = [
        stage_pop_shard(population[o * OUT_C:(o + 1) * OUT_C])
        for o in range(OC_SHARDS)
    ]
    in_maps = []
    for c in range(N_CORES):
        ocs, th = c % OC_SHARDS, c // OC_SHARDS
        in_maps.append({"xT": x_halves[th], "pop": pop_shards[ocs]})
    return in_maps


def assemble(results):
    """Per-core y [Q, 128, TOK_C] bf16 -> full (4, 4096, OUT_F) f32."""
    Y = np.empty((OUT_F, TOKENS), dtype=np.float32)
    for c, r in enumerate(results):
        ocs, th = c % OC_SHARDS, c // OC_SHARDS
        yc = r["y"].astype(np.float32).reshape(OUT_C, TOK_C)
        Y[ocs * OUT_C:(ocs + 1) * OUT_C, th * TOK_C:(th + 1) * TOK_C] = yc
    return np.ascontiguousarray(Y.T).reshape(4, TOKENS // 4, OUT_F)


def kernel(x: np.ndarray, population: np.ndarray):
    in_maps = prep_inputs(x, population)
    nc = _get_nc()
    res = run_bass_kernel_spmd(nc, in_maps, core_ids=list(range(N_CORES)))
    return assemble(res.results)
